# revision 55
# baseline (speedup 1.0000x reference)
"""GQA attention (B=4, T=2048, Hq=16, Hkv=4, hd=128, D=2048) on 8 trn2 cores.

Sharding: core c = (batch b = c//2, row-parity r = c%2). Each core computes
attention for batch b on query row-tiles {2t + r : t in 0..7} (interleaved
128-row tiles, which balances causal work across the two cores of a batch)
and the full output projection for those rows. K/V are computed for the full
sequence on both cores of a batch (cheap), so no cross-core communication is
needed; the host just concatenates disjoint output row slices.

Per-core kernel layouts (everything transposed so no on-device transposes):
  QT/KT:  [head_dim, tok]  (from matmul(lhsT=w_T_tile, rhs=hidden_T_tile))
  V:      [tok, head_dim]  (natural)
  S^T:    [k, q] = KT_tile.T @ QT  -> exp -> P^T
  attn^T: [d, q] = V.T @ P^T       (lhsT=V, rhs=P^T)
  out:    [tok, D] = attn^T.T @ woT
RoPE rotate_half is a signed 128x128 permutation applied with one bf16
matmul; the 1/sqrt(128) query scale is folded into wq host-side. Softmax
skips max-subtraction (scores are O(10) for this distribution) and uses a
ones-column matmul for the partition-axis denominator sums, with the
reciprocal broadcast + normalization inlined per (head, q-half) group
(DVE reciprocal -> GPSIMD partition broadcast -> DVE scale), so no
epilogue pass. Causal masking is multiplicative {0,1} on exp(S^T) at the
diagonal junction only; work on fully-masked regions is skipped via
suffix q-spans. Phase-2 inputs (hq, cos/sin) prefetch during phase 1 and
wo prefetches during phase 3, overlapping DMA with compute.
"""

import numpy as np
import ml_dtypes

import concourse.bass as bass
import concourse.mybir as mybir
import concourse.tile as tile
from concourse import bacc
from concourse.bass_utils import run_bass_kernel_spmd

F32 = mybir.dt.float32
BF16 = mybir.dt.bfloat16
AF = mybir.ActivationFunctionType
NPBF16 = ml_dtypes.bfloat16

P = 128      # partitions / head_dim / row-tile
T = 2048     # full seq len per batch
TQ = 1024    # query rows per core
NH = 16      # query heads
NKV = 4      # kv heads
D = 2048     # model dim
DT = D // P  # 16 D-tiles
CH = 512     # phase-1 token chunk
NCH = T // CH
VPC = CH // P  # V row-tiles per chunk
N_CORES = 8


def build_program(phases=(1, 2, 3, 4), rep=1, interleave2=False,
                  bcast="gpsimd"):
    nc = bacc.Bacc(
        "TRN2", target_bir_lowering=False, debug=False, enable_asserts=False
    )

    def din(name, shape, dt=BF16):
        return nc.dram_tensor(name, shape, dt, kind="ExternalInput").ap()

    # weight/activation layouts are [partition, d-tile, cols] host-side so
    # each tensor loads in 1-4 large DMAs (the ~625ns/DMA HWDGE fixed cost
    # makes many small DMAs issue-bound, ~130 GB/s vs ~345 GB/s)
    ht = din("ht", [P, DT, T])        # hidden[b].T          (for K/V proj)
    hq = din("hq", [P, DT, TQ])       # own-rows hidden.T    (for Q proj)
    wqt = din("wqt", [P, DT, NH * P])  # wq.T, pre-scaled by 1/sqrt(128)
    wkt = din("wkt", [P, DT, NKV * P])  # wk.T
    wvt = din("wvt", [P, DT, NKV * P])  # wv.T
    wot = din("wot", [P, DT, D])      # wo.T
    ck = din("ck", [P, T], F32)       # cos.T full seq
    sk = din("sk", [P, T], F32)
    maskab = din("maskab", [P, 2, P])  # junction masks for (even j, odd j)
    pm = din("pm", [P, P])            # signed rotate_half permutation (bf16)
    onesc = din("onesc", [P, 1])      # ones column (denominator lhsT)
    tick = din("tick", [1, 8], F32)   # timing-chain passthrough

    out = nc.dram_tensor("out", [TQ, D], F32, kind="ExternalOutput").ap()
    tock = nc.dram_tensor("tock", [1, 8], F32, kind="ExternalOutput").ap()

    with tile.TileContext(nc) as tc:
        for rp in range(rep):
            _emit(nc, tc, ht, hq, wqt, wkt, wvt, wot, ck, sk,
                  maskab, pm, onesc, tick, out, tock, phases=phases,
                  pfx=f"_r{rp}" if rep > 1 else "", interleave2=interleave2,
                  bcast=bcast)
    nc.compile()
    return nc


def _emit(nc, tc, ht, hq, wqt, wkt, wvt, wot, ck, sk,
          maskab, pm, onesc, tick, out, tock, phases=(1, 2, 3, 4), pfx="",
          interleave2=True, bcast="gpsimd"):
    from contextlib import ExitStack

    with ExitStack() as ctx:
        # ---- persistent tiles (live across phases) ----
        pers = ctx.enter_context(tc.tile_pool(name="pers" + pfx, bufs=1))
        KT = pers.tile([P, NKV, T], BF16, tag="KT")     # (d, kvh, k) rope'd
        Vsb = pers.tile([P, DT, NKV * P], BF16, tag="V")  # (k%128, ktile, dv)
        QT = pers.tile([P, NH, TQ], BF16, tag="QT")     # (d, h, q) rope'd+scaled
        # normalized attention overwrites QT in place: every write to
        # ATT[:, h, qhalf] is data-dependent on all QT[:, h, qhalf] reads
        # (scores -> exp -> AV -> drain), so aliasing is race-free and saves
        # 4MB of SBUF
        ATT = QT
        pm_sb = pers.tile([P, P], BF16, tag="pm")
        ones_sb = pers.tile([P, 1], BF16, tag="ones")
        mask_sb = pers.tile([P, 2, P], BF16, tag="mask")
        tick_sb = pers.tile([1, 8], F32, tag="tick")
        tock_sb = pers.tile([1, 8], F32, tag="tock")

        def emit_tock():
            # tock = tick passthrough; reps serialize via SBUF pool reuse, so
            # this does not need to trail the output stores — emit it early,
            # off the critical path
            nc.vector.tensor_tensor(tock_sb[:], tick_sb[:], tick_sb[:],
                                    mybir.AluOpType.bypass)
            nc.sync.dma_start(tock[:], tock_sb[:])

        # (small-tensor DMAs are issued inside phase 1, after the startup-
        # critical wk/ht tiles, so the first matmul isn't queued behind them)

        # phase-2 inputs, prefetched during phase 1 (own pool => no WAR wait);
        # released after phase 2 so its space hosts wo during phases 3/4
        p2pre = tc.alloc_tile_pool(name="p2pre" + pfx, bufs=1)
        hqs = p2pre.tile([P, DT, TQ], BF16, tag="hq")
        ck_sb = p2pre.tile([P, DT, P], F32, tag="ck")
        sk_sb = p2pre.tile([P, DT, P], F32, tag="sk")
        cq_sb = p2pre.tile([P, 8, P], F32, tag="cq")   # own-row cos (gathered)
        sq_sb = p2pre.tile([P, 8, P], F32, tag="sq")
        wq0_sb = p2pre.tile([P, DT, 512], BF16, tag="wq0")  # first head quarter

        # timing-variant support: constants normally loaded inside phase 1
        if 1 not in phases:
            nc.sync.dma_start(pm_sb[:], pm[:])
            nc.sync.dma_start(ones_sb[:], onesc[:])
            nc.sync.dma_start(mask_sb[:], maskab[:])
            nc.sync.dma_start(tick_sb[:], tick[:])
            emit_tock()
            if 2 in phases:
                nc.sync.dma_start(wq0_sb[:], wqt[:, :, 0:512])
                nc.sync.dma_start(hqs[:], hq[:])
        # zero tiles whose producer phase is skipped
        if 1 not in phases and 3 in phases:
            nc.any.memzero(KT[:])
            nc.any.memzero(Vsb[:])
        if 2 not in phases and 3 in phases:
            nc.any.memzero(QT[:])
        if 3 not in phases and 4 in phases:
            nc.any.memzero(ATT[:])

        # ================= phase 1: K/V projections + K RoPE =================
        # hidden.T is streamed in 256-token chunks (double-buffered); wk/wv
        # stay resident. Per chunk: K proj for 4 kv heads + RoPE, then V proj
        # for the PREVIOUS chunk's 2 token-tiles (one-chunk lag gives the wv
        # DMA time to land at startup without stalling PE).
        if 1 in phases:
          with tc.tile_pool(name="p1" + pfx, bufs=1) as p1, \
             tc.tile_pool(name="p1h" + pfx, bufs=2) as p1h, \
             tc.tile_pool(name="p1t" + pfx, bufs=2) as p1t, \
             tc.tile_pool(name="ps1" + pfx, bufs=1, space="PSUM") as ps1:
            wks = p1.tile([P, DT, NKV * P], BF16, tag="wk")
            wvs = p1.tile([P, DT, NKV * P], BF16, tag="wv")

            def k_rope(kvh, c, ktmp):
                # rotate via Pm matmul (bf16), combine with cos/sin on DVE
                tsl = slice(c * CH, (c + 1) * CH)
                csl = slice(VPC * c, VPC * (c + 1))
                rot = ps1.tile([P, CH], F32, tag="rot", bufs=2)
                nc.tensor.matmul(rot[:], lhsT=pm_sb[:], rhs=ktmp[:],
                                 start=True, stop=True)
                t2 = p1t.tile([P, CH], BF16, tag="t2")
                nc.vector.tensor_mul(t2[:], rot[:], sk_sb[:, csl, :])
                nc.vector.tensor_mul(ktmp[:], ktmp[:], ck_sb[:, csl, :])
                nc.vector.tensor_add(KT[:, kvh, tsl], ktmp[:], t2[:])

            hts_tiles = {}
            pend = None  # (kvh, c, ktmp) whose RoPE is not yet emitted

            def v_proj(c, hts):
                nonlocal pend
                for v in range(VPC):
                    vt = c * VPC + v
                    vps = ps1.tile([P, NKV * P], F32, tag="vps", bufs=2)
                    for dt in range(DT):
                        nc.tensor.matmul(
                            vps[:],
                            lhsT=hts[:, dt, v * P:(v + 1) * P],
                            rhs=wvs[:, dt, :],
                            start=(dt == 0), stop=(dt == DT - 1))
                    if pend is not None:
                        k_rope(*pend)
                        pend = None
                    nc.vector.tensor_copy(Vsb[:, vt, :], vps[:])

            for c in range(NCH):
                tsl = slice(c * CH, (c + 1) * CH)
                hts = p1h.tile([P, DT, CH], BF16, tag="ht")
                hts_tiles[c] = hts
                if c == 0:
                    # startup-critical interleave in 4-dtile blocks: the
                    # first K matmuls wait only on the first (wk, ht) block;
                    # pm early for the first RoPE rotation matmul. ck/sk DMAs
                    # must be EMITTED here, before the first k_rope reader —
                    # the tile framework derives dependencies from emission
                    # order, so a later-emitted producer DMA becomes a racy
                    # WAR instead of the RAW the reader needs.
                    for q4 in range(4):
                        sl = slice(4 * q4, 4 * q4 + 4)
                        nc.sync.dma_start(wks[:, sl], wkt[:, sl])
                        nc.sync.dma_start(hts[:, sl], ht[:, sl, tsl])
                        if q4 == 0:
                            nc.sync.dma_start(pm_sb[:], pm[:])
                    nc.sync.dma_start(ck_sb[:], ck[:])
                    nc.sync.dma_start(sk_sb[:], sk[:])
                else:
                    nc.sync.dma_start(hts[:], ht[:, :, tsl])
                # spread the non-immediate loads so early ht chunks aren't
                # starved: wv over chunks 0-1 (first V proj runs in iter 1),
                # hq prefetch over 1-2
                if c in (0, 1):
                    sl = slice(8 * c, 8 * c + 8)
                    nc.sync.dma_start(wvs[:, sl], wvt[:, sl])
                if c == 1:
                    nc.sync.dma_start(ones_sb[:], onesc[:])
                    nc.sync.dma_start(mask_sb[:], maskab[:])
                    nc.sync.dma_start(tick_sb[:], tick[:])
                    emit_tock()
                if c in (1, 2):
                    for g in range(2):
                        sl = slice(8 * (c - 1) + 4 * g, 8 * (c - 1) + 4 * g + 4)
                        nc.sync.dma_start(hqs[:, sl], hq[:, sl])
                if c == 2:
                    nc.sync.dma_start(wq0_sb[:], wqt[:, :, 0:512])
                # K projection for this chunk; the RoPE rotation matmul of the
                # previous (kvh, chunk) issues between projections so PE never
                # waits on the ACT PSUM->SBUF copy.
                for kvh in range(NKV):
                    kps = ps1.tile([P, CH], F32, tag="kps", bufs=2)
                    for dt in range(DT):
                        nc.tensor.matmul(
                            kps[:],
                            lhsT=wks[:, dt, kvh * P:(kvh + 1) * P],
                            rhs=hts[:, dt, :],
                            start=(dt == 0), stop=(dt == DT - 1))
                    ktmp = p1t.tile([P, CH], BF16, tag="ktmp")
                    nc.scalar.activation(ktmp[:], kps[:], AF.Copy)
                    if pend is not None:
                        k_rope(*pend)
                    pend = (kvh, c, ktmp)
                # V projection for the previous chunk (its hts is still live
                # in the double buffer)
                if c > 0:
                    v_proj(c - 1, hts_tiles.pop(c - 1))
            v_proj(NCH - 1, hts_tiles.pop(NCH - 1))
            if pend is not None:
                k_rope(*pend)
                pend = None

        # Own-row cos/sin for Q RoPE: the program is SPMD-shared, so the
        # gather pattern (even 128-col blocks of ck/sk) is fixed; odd-parity
        # cores receive ht/ck/sk with adjacent 128-token blocks swapped
        # host-side so their own rows land in the even slots (see
        # make_in_maps -- the pair swap permutes k-tiles, which is reduction-
        # order-invariant, and the junction masks account for the slot swap).
        if 2 in phases:
            if 1 not in phases:
                nc.sync.dma_start(ck_sb[:], ck[:])
                nc.sync.dma_start(sk_sb[:], sk[:])
            nc.sync.dma_start(cq_sb[:], ck_sb[:, 0::2, :])
            nc.sync.dma_start(sq_sb[:], sk_sb[:, 0::2, :])

        # ================= phase 2: Q projection + RoPE =================
        # wq.T streamed in 4-head quarters (double-buffered); hq prefetched.
        if 2 in phases:
          with tc.tile_pool(name="p2w" + pfx, bufs=2) as p2w, \
             tc.tile_pool(name="p2t" + pfx, bufs=2) as p2t, \
             tc.tile_pool(name="ps2" + pfx, bufs=1, space="PSUM") as ps2:
            def q_rope(h, qtmp):
                rot = ps2.tile([P, TQ], F32, tag="qrot", bufs=2)
                nc.tensor.matmul(rot[:, 0:512], lhsT=pm_sb[:],
                                 rhs=qtmp[:, 0:512], start=True, stop=True)
                nc.tensor.matmul(rot[:, 512:1024], lhsT=pm_sb[:],
                                 rhs=qtmp[:, 512:1024], start=True, stop=True)
                t2 = p2t.tile([P, TQ], BF16, tag="qt2")
                nc.vector.tensor_mul(t2[:], rot[:], sq_sb[:])
                nc.vector.tensor_mul(qtmp[:], qtmp[:], cq_sb[:])
                nc.vector.tensor_add(QT[:, h, :], qtmp[:], t2[:])

            pend = None  # (h, qtmp) whose RoPE is not yet emitted
            for g in range(4):               # head quarters
                if g == 0:
                    wq_sb = wq0_sb   # prefetched during phase 1
                else:
                    wq_sb = p2w.tile([P, DT, 512], BF16, tag="wq")
                    nc.sync.dma_start(wq_sb[:],
                                      wqt[:, :, g * 512:(g + 1) * 512])
                for hh in range(4):
                    h = g * 4 + hh
                    qps = ps2.tile([P, TQ], F32, tag="qps", bufs=2)
                    for dt in range(DT):
                        for nb in range(2):
                            nc.tensor.matmul(
                                qps[:, nb * 512:(nb + 1) * 512],
                                lhsT=wq_sb[:, dt, hh * P:(hh + 1) * P],
                                rhs=hqs[:, dt, nb * 512:(nb + 1) * 512],
                                start=(dt == 0), stop=(dt == DT - 1))
                    qtmp = p2t.tile([P, TQ], BF16, tag="qtmp")
                    nc.scalar.activation(qtmp[:], qps[:], AF.Copy)
                    if pend is not None:
                        q_rope(*pend)
                    pend = (h, qtmp)
            q_rope(*pend)
        p2pre.release()

        # ================= phase 3: causal attention (transposed) =============
        # Local q-tile t covers global row-tile g = 2t + r; it attends to
        # k-tiles j <= 2t + 1 (the odd-parity core's diagonal; the even core
        # wastes the last one, fully masked via mask data). For k-tile j the
        # attending q suffix starts at local tile j//2.
        # Normalization is inlined per (h, qh) group: DVE reciprocal of the
        # PSUM denominator row, GPSIMD broadcast across partitions, DVE scale
        # of the PSUM AV accumulator directly into ATT (bf16).
        if 3 in phases or 4 in phases:
            p34 = ctx.enter_context(tc.tile_pool(name="p34" + pfx, bufs=1))
            wo_sb = p34.tile([P, DT, D], BF16, tag="wo")
            for q4 in range(4):
                sl = slice(4 * q4, 4 * q4 + 4)
                nc.sync.dma_start(wo_sb[:, sl], wot[:, sl])

        if 3 in phases:
          with tc.tile_pool(name="p3t" + pfx, bufs=1) as p3t, \
             tc.tile_pool(name="p3d" + pfx, bufs=1, space="DRAM") as p3d, \
             tc.tile_pool(name="ps3" + pfx, bufs=1, space="PSUM") as ps3:
            # One flat software pipeline over all (h, qh, pair) tasks:
            # scores+exp for task i issue before AV/ones for task i-1, so PE
            # never waits on ACT's exp. Tasks of HEAD PAIRS are interleaved
            # (A0 B0 A1 B1 ...) so each head's consecutive tasks sit two
            # pipeline slots apart — doubling the slack available to hide
            # exp latency and cross-engine semaphores. Within each (h, qh)
            # group the short suffix tasks run mid-group (descending tail) so
            # groups end on wide tasks. The widest task must stay first: its
            # AV accumulation window covers all later (narrower) windows.
            # PSUM: stp 2x2 banks + avp 3x1 + all dnp rows packed into one
            # bank at base partitions {0, 32, 64} = 8 banks.
            def group_ps(qh):
                return [0, 3, 2, 1] if qh == 0 else [0, 1, 2, 3, 4, 7, 6, 5]

            def head_tasks(h):
                return [(h, qh, p, pos) for qh in (0, 1)
                        for pos, p in enumerate(group_ps(qh))]
            tasks = []
            if interleave2:
                for i in range(NH // 2):
                    for a, b in zip(head_tasks(2 * i), head_tasks(2 * i + 1)):
                        tasks.append(a)
                        tasks.append(b)
            else:
                for h in range(NH):
                    tasks.extend(head_tasks(h))
            state = {}           # (h, qh) -> (avp, dnp)
            prev = None          # (h, qh, p, pt)
            for tsk in tasks + [None]:
                if tsk is not None:
                    h, qh, p, pos = tsk
                    kvh = h // 4
                    qbase = qh * 512
                    if pos == 0:
                        avp_new = ps3.tile([P, 512], F32, tag="av", bufs=2,
                                           name=f"avp{pfx}_{h}_{qh}")
                        dnp_new = ps3.tile([1, 512], F32, tag="dn", bufs=2,
                                           name=f"dnp{pfx}_{h}_{qh}")
                        state[(h, qh)] = (avp_new, dnp_new)
                    qs = max(0, p - 4 * qh) * P
                    stp = ps3.tile([P, 2, 512], F32, tag="st", bufs=2)
                    for jj in range(2):
                        j = 2 * p + jj
                        nc.tensor.matmul(
                            stp[:, jj, qs:512],
                            lhsT=KT[:, kvh, j * P:(j + 1) * P],
                            rhs=QT[:, h, qbase + qs:qbase + 512],
                            start=True, stop=True)
                    pt = p3t.tile([P, 2, 512], BF16, tag="pt", bufs=4)
                    nc.scalar.activation(pt[:, :, qs:512],
                                         stp[:, :, qs:512], AF.Exp)
                    if p >= 4 * qh:      # diagonal junction: causal mask
                        nc.vector.tensor_mul(pt[:, :, qs:qs + P],
                                             pt[:, :, qs:qs + P], mask_sb[:])
                    # pair-sum on DVE halves the PE denominator matmul work
                    pts = p3t.tile([P, 512], BF16, tag="pts", bufs=3)
                    nc.vector.tensor_add(pts[:, qs:512], pt[:, 0, qs:512],
                                         pt[:, 1, qs:512])
                    prev_pts = pts
                if prev is not None:
                    ph, pqh, pp, ppos, ppt, ppts = prev
                    pkvh = ph // 4
                    NP = 4 * (pqh + 1)
                    pqs = max(0, pp - 4 * pqh) * P
                    avp, dnp = state[(ph, pqh)]
                    for jj in range(2):
                        j = 2 * pp + jj
                        nc.tensor.matmul(
                            avp[:, pqs:512],
                            lhsT=Vsb[:, j, pkvh * P:(pkvh + 1) * P],
                            rhs=ppt[:, jj, pqs:512],
                            start=(ppos == 0 and jj == 0),
                            stop=(ppos == NP - 1 and jj == 1))
                    nc.tensor.matmul(
                        dnp[:, pqs:512],
                        lhsT=ones_sb[:],
                        rhs=ppts[:, pqs:512],
                        start=(ppos == 0), stop=(ppos == NP - 1))
                    if ppos == NP - 1:   # last pair of (ph, pqh): drain
                        pqbase = pqh * 512
                        rcp = p3t.tile([1, 512], F32, tag="rcp", bufs=2)
                        scr = p3t.tile([1, 512], F32, tag="scr", bufs=2)
                        nc.vector.reciprocal_approx_accurate(
                            rcp[:], dnp[:], scr[:])
                        rdb = p3t.tile([P, 512], F32, tag="rdb", bufs=2)
                        if bcast == "gpsimd":
                            nc.gpsimd.partition_broadcast(rdb[:], rcp[:])
                        else:
                            # DRAM-bounce broadcast (DMA engines only)
                            rcpd = p3d.tile([1, 512], F32, tag="rcpd", bufs=2)
                            nc.sync.dma_start(rcpd[:], rcp[:])
                            nc.sync.dma_start(
                                rdb[:], rcpd[0:1, :].to_broadcast((P, 512)))
                        nc.vector.tensor_mul(
                            ATT[:, ph, pqbase:pqbase + 512], avp[:], rdb[:])
                        del state[(ph, pqh)]
                prev = (h, qh, p, pos, pt, prev_pts) if tsk is not None else None

        # ================= phase 4: output projection =================
        if 4 in phases:
          with tc.tile_pool(name="p4" + pfx, bufs=1) as p4, \
             tc.tile_pool(name="ps4" + pfx, bufs=1, space="PSUM") as ps4:
            for tt in range(TQ // P):
                for cb in range(2):
                    last = (tt == TQ // P - 1 and cb == 1)
                    if not last:
                        ops = ps4.tile([P, 1024], F32, tag="ops", bufs=2)
                        for htile in range(NH):
                            for nb in range(2):
                                nc.tensor.matmul(
                                    ops[:, nb * 512:(nb + 1) * 512],
                                    lhsT=ATT[:, htile, tt * P:(tt + 1) * P],
                                    rhs=wo_sb[:, htile,
                                              cb * 1024 + nb * 512:cb * 1024 + (nb + 1) * 512],
                                    start=(htile == 0), stop=(htile == NH - 1))
                        osb = p4.tile([P, 1024], F32, tag="osb", bufs=3)
                        nc.scalar.activation(osb[:], ops[:], AF.Copy)
                        nc.sync.dma_start(
                            out[tt * P:(tt + 1) * P, cb * 1024:(cb + 1) * 1024],
                            osb[:])
                    else:
                        # finer tail: the final tile runs as four independent
                        # 256-col accumulation groups so copy+store drain as
                        # each group completes, shortening the post-PE tail
                        for nb in range(4):
                            opsl = ps4.tile([P, 256], F32, tag="opsl", bufs=2)
                            csl = slice(cb * 1024 + nb * 256,
                                        cb * 1024 + (nb + 1) * 256)
                            for htile in range(NH):
                                nc.tensor.matmul(
                                    opsl[:],
                                    lhsT=ATT[:, htile, tt * P:(tt + 1) * P],
                                    rhs=wo_sb[:, htile, csl],
                                    start=(htile == 0), stop=(htile == NH - 1))
                            osb = p4.tile([P, 256], F32, tag="osbl", bufs=4)
                            nc.scalar.activation(osb[:], opsl[:], AF.Copy)
                            nc.sync.dma_start(
                                out[tt * P:(tt + 1) * P, csl], osb[:])


# ---------------------------------------------------------------------------
# host-side wrapper
# ---------------------------------------------------------------------------

_NC = None


def _get_nc():
    global _NC
    if _NC is None:
        _NC = build_program()
    return _NC


def make_in_maps(hidden_states, cos, sin, wq, wk, wv, wo):
    """Build the 8 per-core input dicts (host-side sharding/layout prep).

    Odd-parity cores receive ht/ck/sk with adjacent 128-token blocks swapped
    so their own rows sit in even slots (the SPMD-shared on-device cq/sq
    gather reads even blocks). The swap permutes k-tiles within each pair,
    which only reorders the attention reduction; at the causal junction the
    diagonal lands in the even slot for BOTH parities, with the odd slot
    fully masked (even cores, waste tile) or fully attended (odd cores).
    The 1/sqrt(128) query scale is folded into wqt.
    """
    scale = np.float32(1.0 / np.sqrt(P))

    def tiled(w):
        # [D, N] -> [P, DT, N] (partition-major d-tiles for single-DMA loads)
        wT = np.asarray(w).T
        return np.ascontiguousarray(
            wT.reshape(DT, P, wT.shape[1]).transpose(1, 0, 2)).astype(NPBF16)

    wqt = tiled(np.asarray(wq).astype(np.float64) * scale)
    wkt = tiled(wk)
    wvt = tiled(wv)
    wot = tiled(wo)
    pmat = np.zeros((P, P), np.float32)
    for m in range(64):
        pmat[m + 64, m] = -1.0      # out[m] = -in[m+64]
        pmat[m, m + 64] = 1.0       # out[m+64] = in[m]
    onesc = np.ones((P, 1), NPBF16)
    tri = (np.arange(P)[:, None] <= np.arange(P)[None, :])  # [k, q]: k <= q

    in_maps = []
    for c in range(N_CORES):
        b, r = c // 2, c % 2
        hb = np.asarray(hidden_states[b])                   # [T, D] f32
        own = hb.reshape(T // P, P, D)[r::2].reshape(TQ, D)
        cosb = np.asarray(cos[b])                           # [T, 128]
        sinb = np.asarray(sin[b])
        if r == 1:
            # swap adjacent 128-token blocks so own rows sit in EVEN slots
            swp = np.arange(T // P).reshape(-1, 2)[:, ::-1].reshape(-1)
            hb_k = hb.reshape(T // P, P, D)[swp].reshape(T, D)
            cos_k = cosb.reshape(T // P, P, P)[swp].reshape(T, P)
            sin_k = sinb.reshape(T // P, P, P)[swp].reshape(T, P)
        else:
            hb_k, cos_k, sin_k = hb, cosb, sinb
        maskab_c = np.empty((P, 2, P), np.float32)
        # diagonal in the even slot for both parities; odd slot is the waste
        # tile (even cores) or a fully-attended earlier tile (odd cores)
        maskab_c[:, 0, :] = tri
        maskab_c[:, 1, :] = 0.0 if r == 0 else 1.0
        in_maps.append({
            "ht": tiled(hb_k),
            "hq": tiled(own),
            "wqt": wqt, "wkt": wkt, "wvt": wvt, "wot": wot,
            "ck": np.ascontiguousarray(cos_k.T),
            "sk": np.ascontiguousarray(sin_k.T),
            "maskab": maskab_c.astype(NPBF16),
            "pm": pmat.astype(NPBF16),
            "onesc": onesc,
            "tick": np.zeros((1, 8), np.float32),
        })
    return in_maps


def assemble_output(results):
    out = np.empty((4, T, D), np.float32)
    for c in range(N_CORES):
        b, r = c // 2, c % 2
        out[b].reshape(T // P, P, D)[r::2] = results[c]["out"].reshape(TQ // P, P, D)
    return out


def kernel(hidden_states, cos, sin, wq, wk, wv, wo):
    nc = _get_nc()
    in_maps = make_in_maps(hidden_states, cos, sin, wq, wk, wv, wo)
    res = run_bass_kernel_spmd(nc, in_maps, list(range(N_CORES)))
    return assemble_output(res.results)


if __name__ == "__main__":
    rng = np.random.default_rng(0)
    args = {
        "hidden_states": rng.standard_normal((4, T, D), np.float32),
        "cos": rng.random((4, T, P), np.float32),
        "sin": rng.random((4, T, P), np.float32),
        "wq": rng.standard_normal((NH * P, D), np.float32) / np.sqrt(D),
        "wk": rng.standard_normal((NKV * P, D), np.float32) / np.sqrt(D),
        "wv": rng.standard_normal((NKV * P, D), np.float32) / np.sqrt(D),
        "wo": rng.standard_normal((D, NH * P), np.float32) / np.sqrt(D),
    }
    o = kernel(**args)
    print("ran:", o.shape, o.dtype, np.abs(o).max())


# revision 59
# speedup vs baseline: 1.0375x; 1.0375x over previous
"""GQA attention (B=4, T=2048, Hq=16, Hkv=4, hd=128, D=2048) on 8 trn2 cores.

Sharding: core c = (batch b = c//2, row-parity r = c%2). Each core computes
attention for batch b on query row-tiles {2t + r : t in 0..7} (interleaved
128-row tiles, which balances causal work across the two cores of a batch)
and the full output projection for those rows. K/V are computed for the full
sequence on both cores of a batch (cheap), so no cross-core communication is
needed; the host just concatenates disjoint output row slices.

Per-core kernel layouts (everything transposed so no on-device transposes):
  QT/KT:  [head_dim, tok]  (from matmul(lhsT=w_T_tile, rhs=hidden_T_tile))
  V:      [tok, head_dim]  (natural)
  S^T:    [k, q] = KT_tile.T @ QT  -> exp -> P^T
  attn^T: [d, q] = V.T @ P^T       (lhsT=V, rhs=P^T)
  out:    [tok, D] = attn^T.T @ woT
RoPE rotate_half is a signed 128x128 permutation applied with one bf16
matmul; the 1/sqrt(128) query scale is folded into wq host-side. Softmax
skips max-subtraction (scores are O(10) for this distribution) and uses a
ones-column matmul for the partition-axis denominator sums, with the
reciprocal broadcast + normalization inlined per (head, q-half) group
(DVE reciprocal -> GPSIMD partition broadcast -> DVE scale), so no
epilogue pass. Causal masking is multiplicative {0,1} on exp(S^T) at the
diagonal junction only; work on fully-masked regions is skipped via
suffix q-spans. Phase-2 inputs (hq, cos/sin) prefetch during phase 1 and
wo prefetches during phase 3, overlapping DMA with compute.
"""

import numpy as np
import ml_dtypes

import concourse.bass as bass
import concourse.mybir as mybir
import concourse.tile as tile
from concourse import bacc
from concourse.bass_utils import run_bass_kernel_spmd

F32 = mybir.dt.float32
BF16 = mybir.dt.bfloat16
AF = mybir.ActivationFunctionType
NPBF16 = ml_dtypes.bfloat16

P = 128      # partitions / head_dim / row-tile
T = 2048     # full seq len per batch
TQ = 1024    # query rows per core
NH = 16      # query heads
NKV = 4      # kv heads
D = 2048     # model dim
DT = D // P  # 16 D-tiles
CH = 512     # phase-1 token chunk
NCH = T // CH
VPC = CH // P  # V row-tiles per chunk
N_CORES = 8


def build_program(phases=(1, 2, 3, 4), rep=1, interleave2=False,
                  bcast="gpsimd"):
    nc = bacc.Bacc(
        "TRN2", target_bir_lowering=False, debug=False, enable_asserts=False
    )

    def din(name, shape, dt=BF16):
        return nc.dram_tensor(name, shape, dt, kind="ExternalInput").ap()

    # weight/activation layouts are [partition, d-tile, cols] host-side so
    # each tensor loads in 1-4 large DMAs (the ~625ns/DMA HWDGE fixed cost
    # makes many small DMAs issue-bound, ~130 GB/s vs ~345 GB/s)
    ht = din("ht", [P, DT, T])        # hidden[b].T          (for K/V proj)
    hq = din("hq", [P, DT, TQ])       # own-rows hidden.T    (for Q proj)
    wqt = din("wqt", [P, DT, NH * P])  # wq.T, pre-scaled by 1/sqrt(128)
    wkt = din("wkt", [P, DT, NKV * P])  # wk.T
    wvt = din("wvt", [P, DT, NKV * P])  # wv.T
    wot = din("wot", [P, DT, D])      # wo.T
    ck = din("ck", [P, T], F32)       # cos.T full seq
    sk = din("sk", [P, T], F32)
    maskab = din("maskab", [P, 2, P])  # junction masks for (even j, odd j)
    pm = din("pm", [P, P])            # signed rotate_half permutation (bf16)
    onesc = din("onesc", [P, 1])      # ones column (denominator lhsT)
    tick = din("tick", [1, 8], F32)   # timing-chain passthrough

    out = nc.dram_tensor("out", [TQ, D], F32, kind="ExternalOutput").ap()
    tock = nc.dram_tensor("tock", [1, 8], F32, kind="ExternalOutput").ap()

    with tile.TileContext(nc) as tc:
        for rp in range(rep):
            _emit(nc, tc, ht, hq, wqt, wkt, wvt, wot, ck, sk,
                  maskab, pm, onesc, tick, out, tock, phases=phases,
                  pfx=f"_r{rp}" if rep > 1 else "", interleave2=interleave2,
                  bcast=bcast)
    nc.compile()
    return nc


def _emit(nc, tc, ht, hq, wqt, wkt, wvt, wot, ck, sk,
          maskab, pm, onesc, tick, out, tock, phases=(1, 2, 3, 4), pfx="",
          interleave2=True, bcast="gpsimd"):
    from contextlib import ExitStack

    with ExitStack() as ctx:
        # ---- persistent tiles (live across phases) ----
        pers = ctx.enter_context(tc.tile_pool(name="pers" + pfx, bufs=1))
        KT = pers.tile([P, NKV, T], BF16, tag="KT")     # (d, kvh, k) rope'd
        Vsb = pers.tile([P, DT, NKV * P], BF16, tag="V")  # (k%128, ktile, dv)
        QT = pers.tile([P, NH, TQ], BF16, tag="QT")     # (d, h, q) rope'd+scaled
        # normalized attention overwrites QT in place: every write to
        # ATT[:, h, qhalf] is data-dependent on all QT[:, h, qhalf] reads
        # (scores -> exp -> AV -> drain), so aliasing is race-free and saves
        # 4MB of SBUF
        ATT = QT
        pm_sb = pers.tile([P, P], BF16, tag="pm")
        ones_sb = pers.tile([P, 1], BF16, tag="ones")
        mask_sb = pers.tile([P, 2, P], BF16, tag="mask")
        tick_sb = pers.tile([1, 8], F32, tag="tick")
        tock_sb = pers.tile([1, 8], F32, tag="tock")

        def emit_tock():
            # tock = tick passthrough; reps serialize via SBUF pool reuse, so
            # this does not need to trail the output stores — emit it early,
            # off the critical path
            nc.vector.tensor_tensor(tock_sb[:], tick_sb[:], tick_sb[:],
                                    mybir.AluOpType.bypass)
            nc.sync.dma_start(tock[:], tock_sb[:])

        # (small-tensor DMAs are issued inside phase 1, after the startup-
        # critical wk/ht tiles, so the first matmul isn't queued behind them)

        # phase-2 inputs, prefetched during phase 1 (own pool => no WAR wait);
        # released after phase 2 so its space hosts wo during phases 3/4
        p2pre = tc.alloc_tile_pool(name="p2pre" + pfx, bufs=1)
        hqs = p2pre.tile([P, DT, TQ], BF16, tag="hq")
        ck_sb = p2pre.tile([P, DT, P], F32, tag="ck")
        sk_sb = p2pre.tile([P, DT, P], F32, tag="sk")
        cq_sb = p2pre.tile([P, 8, P], F32, tag="cq")   # own-row cos (gathered)
        sq_sb = p2pre.tile([P, 8, P], F32, tag="sq")
        wq0_sb = p2pre.tile([P, DT, 512], BF16, tag="wq0")  # first head quarter

        # timing-variant support: constants normally loaded inside phase 1
        if 1 not in phases:
            nc.sync.dma_start(pm_sb[:], pm[:])
            nc.sync.dma_start(ones_sb[:], onesc[:])
            nc.sync.dma_start(mask_sb[:], maskab[:])
            nc.sync.dma_start(tick_sb[:], tick[:])
            emit_tock()
            if 2 in phases:
                nc.sync.dma_start(wq0_sb[:], wqt[:, :, 0:512])
                nc.sync.dma_start(hqs[:], hq[:])
        # zero tiles whose producer phase is skipped
        if 1 not in phases and 3 in phases:
            nc.any.memzero(KT[:])
            nc.any.memzero(Vsb[:])
        if 2 not in phases and 3 in phases:
            nc.any.memzero(QT[:])
        if 3 not in phases and 4 in phases:
            nc.any.memzero(ATT[:])

        # ================= phase 1: K/V projections + K RoPE =================
        # hidden.T is streamed in 256-token chunks (double-buffered); wk/wv
        # stay resident. Per chunk: K proj for 4 kv heads + RoPE, then V proj
        # for the PREVIOUS chunk's 2 token-tiles (one-chunk lag gives the wv
        # DMA time to land at startup without stalling PE).
        if 1 in phases:
          with tc.tile_pool(name="p1" + pfx, bufs=1) as p1, \
             tc.tile_pool(name="p1h" + pfx, bufs=2) as p1h, \
             tc.tile_pool(name="p1t" + pfx, bufs=2) as p1t, \
             tc.tile_pool(name="ps1" + pfx, bufs=1, space="PSUM") as ps1:
            wks = p1.tile([P, DT, NKV * P], BF16, tag="wk")
            wvs = p1.tile([P, DT, NKV * P], BF16, tag="wv")

            def k_rope(kvh, c, ktmp):
                # rotate via Pm matmul (bf16), combine with cos/sin on DVE
                tsl = slice(c * CH, (c + 1) * CH)
                csl = slice(VPC * c, VPC * (c + 1))
                rot = ps1.tile([P, CH], F32, tag="rot", bufs=2)
                nc.tensor.matmul(rot[:], lhsT=pm_sb[:], rhs=ktmp[:],
                                 start=True, stop=True)
                t2 = p1t.tile([P, CH], BF16, tag="t2")
                nc.vector.tensor_mul(t2[:], rot[:], sk_sb[:, csl, :])
                nc.vector.tensor_mul(ktmp[:], ktmp[:], ck_sb[:, csl, :])
                nc.vector.tensor_add(KT[:, kvh, tsl], ktmp[:], t2[:])

            hts_tiles = {}
            pend = None  # (kvh, c, ktmp) whose RoPE is not yet emitted

            def v_proj(c, hts):
                nonlocal pend
                for v in range(VPC):
                    vt = c * VPC + v
                    vps = ps1.tile([P, NKV * P], F32, tag="vps", bufs=2)
                    for dt in range(DT):
                        nc.tensor.matmul(
                            vps[:],
                            lhsT=hts[:, dt, v * P:(v + 1) * P],
                            rhs=wvs[:, dt, :],
                            start=(dt == 0), stop=(dt == DT - 1))
                    if pend is not None:
                        k_rope(*pend)
                        pend = None
                    nc.vector.tensor_copy(Vsb[:, vt, :], vps[:])

            for c in range(NCH):
                tsl = slice(c * CH, (c + 1) * CH)
                hts = p1h.tile([P, DT, CH], BF16, tag="ht")
                hts_tiles[c] = hts
                if c == 0:
                    # startup-critical interleave in 4-dtile blocks: the
                    # first K matmuls wait only on the first (wk, ht) block;
                    # pm early for the first RoPE rotation matmul. ck/sk DMAs
                    # must be EMITTED here, before the first k_rope reader —
                    # the tile framework derives dependencies from emission
                    # order, so a later-emitted producer DMA becomes a racy
                    # WAR instead of the RAW the reader needs.
                    for q4 in range(4):
                        sl = slice(4 * q4, 4 * q4 + 4)
                        nc.sync.dma_start(wks[:, sl], wkt[:, sl])
                        nc.sync.dma_start(hts[:, sl], ht[:, sl, tsl])
                        if q4 == 0:
                            nc.sync.dma_start(pm_sb[:], pm[:])
                    nc.sync.dma_start(ck_sb[:], ck[:])
                    nc.sync.dma_start(sk_sb[:], sk[:])
                else:
                    nc.sync.dma_start(hts[:], ht[:, :, tsl])
                # spread the non-immediate loads so early ht chunks aren't
                # starved: wv over chunks 0-1 (first V proj runs in iter 1),
                # hq prefetch over 1-2
                if c in (0, 1):
                    sl = slice(8 * c, 8 * c + 8)
                    nc.sync.dma_start(wvs[:, sl], wvt[:, sl])
                if c == 1:
                    nc.sync.dma_start(ones_sb[:], onesc[:])
                    nc.sync.dma_start(mask_sb[:], maskab[:])
                    nc.sync.dma_start(tick_sb[:], tick[:])
                    emit_tock()
                if c in (1, 2):
                    for g in range(2):
                        sl = slice(8 * (c - 1) + 4 * g, 8 * (c - 1) + 4 * g + 4)
                        nc.sync.dma_start(hqs[:, sl], hq[:, sl])
                if c == 2:
                    nc.sync.dma_start(wq0_sb[:], wqt[:, :, 0:512])
                # K projection for this chunk; the RoPE rotation matmul of the
                # previous (kvh, chunk) issues between projections so PE never
                # waits on the ACT PSUM->SBUF copy.
                for kvh in range(NKV):
                    kps = ps1.tile([P, CH], F32, tag="kps", bufs=2)
                    for dt in range(DT):
                        nc.tensor.matmul(
                            kps[:],
                            lhsT=wks[:, dt, kvh * P:(kvh + 1) * P],
                            rhs=hts[:, dt, :],
                            start=(dt == 0), stop=(dt == DT - 1))
                    ktmp = p1t.tile([P, CH], BF16, tag="ktmp")
                    nc.scalar.activation(ktmp[:], kps[:], AF.Copy)
                    if pend is not None:
                        k_rope(*pend)
                    pend = (kvh, c, ktmp)
                # V projection for the previous chunk (its hts is still live
                # in the double buffer)
                if c > 0:
                    v_proj(c - 1, hts_tiles.pop(c - 1))
            v_proj(NCH - 1, hts_tiles.pop(NCH - 1))
            if pend is not None:
                k_rope(*pend)
                pend = None

        # Own-row cos/sin for Q RoPE: the program is SPMD-shared, so the
        # gather pattern (even 128-col blocks of ck/sk) is fixed; odd-parity
        # cores receive ht/ck/sk with adjacent 128-token blocks swapped
        # host-side so their own rows land in the even slots (see
        # make_in_maps -- the pair swap permutes k-tiles, which is reduction-
        # order-invariant, and the junction masks account for the slot swap).
        if 2 in phases:
            if 1 not in phases:
                nc.sync.dma_start(ck_sb[:], ck[:])
                nc.sync.dma_start(sk_sb[:], sk[:])
            nc.sync.dma_start(cq_sb[:], ck_sb[:, 0::2, :])
            nc.sync.dma_start(sq_sb[:], sk_sb[:, 0::2, :])

        # ================= phase 2: Q projection + RoPE =================
        # wq.T streamed in 4-head quarters (double-buffered); hq prefetched.
        if 2 in phases:
          with tc.tile_pool(name="p2w" + pfx, bufs=2) as p2w, \
             tc.tile_pool(name="p2t" + pfx, bufs=2) as p2t, \
             tc.tile_pool(name="ps2" + pfx, bufs=1, space="PSUM") as ps2:
            def q_rope(h, qtmp):
                rot = ps2.tile([P, TQ], F32, tag="qrot", bufs=2)
                nc.tensor.matmul(rot[:, 0:512], lhsT=pm_sb[:],
                                 rhs=qtmp[:, 0:512], start=True, stop=True)
                nc.tensor.matmul(rot[:, 512:1024], lhsT=pm_sb[:],
                                 rhs=qtmp[:, 512:1024], start=True, stop=True)
                t2 = p2t.tile([P, TQ], BF16, tag="qt2")
                nc.vector.tensor_mul(t2[:], rot[:], sq_sb[:])
                nc.vector.tensor_mul(qtmp[:], qtmp[:], cq_sb[:])
                nc.vector.tensor_add(QT[:, h, :], qtmp[:], t2[:])

            pend = None  # (h, qtmp) whose RoPE is not yet emitted
            for g in range(4):               # head quarters
                if g == 0:
                    wq_sb = wq0_sb   # prefetched during phase 1
                else:
                    wq_sb = p2w.tile([P, DT, 512], BF16, tag="wq")
                    nc.sync.dma_start(wq_sb[:],
                                      wqt[:, :, g * 512:(g + 1) * 512])
                for hh in range(4):
                    h = g * 4 + hh
                    qps = ps2.tile([P, TQ], F32, tag="qps", bufs=2)
                    for dt in range(DT):
                        for nb in range(2):
                            nc.tensor.matmul(
                                qps[:, nb * 512:(nb + 1) * 512],
                                lhsT=wq_sb[:, dt, hh * P:(hh + 1) * P],
                                rhs=hqs[:, dt, nb * 512:(nb + 1) * 512],
                                start=(dt == 0), stop=(dt == DT - 1))
                    qtmp = p2t.tile([P, TQ], BF16, tag="qtmp")
                    nc.scalar.activation(qtmp[:], qps[:], AF.Copy)
                    if pend is not None:
                        q_rope(*pend)
                    pend = (h, qtmp)
            q_rope(*pend)
        p2pre.release()

        # ================= phase 3: causal attention (transposed) =============
        # Local q-tile t covers global row-tile g = 2t + r; it attends to
        # k-tiles j <= 2t + 1 (the odd-parity core's diagonal; the even core
        # wastes the last one, fully masked via mask data). For k-tile j the
        # attending q suffix starts at local tile j//2.
        # Normalization is inlined per (h, qh) group: DVE reciprocal of the
        # PSUM denominator row, GPSIMD broadcast across partitions, DVE scale
        # of the PSUM AV accumulator directly into ATT (bf16).
        if 3 in phases or 4 in phases:
            p34 = ctx.enter_context(tc.tile_pool(name="p34" + pfx, bufs=1))
            wo_sb = p34.tile([P, DT, D], BF16, tag="wo")
            for q4 in range(4):
                sl = slice(4 * q4, 4 * q4 + 4)
                nc.sync.dma_start(wo_sb[:, sl], wot[:, sl])

        if 3 in phases:
          with tc.tile_pool(name="p3t" + pfx, bufs=1) as p3t, \
             tc.tile_pool(name="p3d" + pfx, bufs=1, space="DRAM") as p3d, \
             tc.tile_pool(name="ps3" + pfx, bufs=1, space="PSUM") as ps3:
            # One flat software pipeline over all (h, qh, pair) tasks:
            # scores+exp for task i issue before AV/ones for task i-1, so PE
            # never waits on ACT's exp. Tasks of HEAD PAIRS are interleaved
            # (A0 B0 A1 B1 ...) so each head's consecutive tasks sit two
            # pipeline slots apart — doubling the slack available to hide
            # exp latency and cross-engine semaphores. Within each (h, qh)
            # group the short suffix tasks run mid-group (descending tail) so
            # groups end on wide tasks. The widest task must stay first: its
            # AV accumulation window covers all later (narrower) windows.
            # PSUM: stp 2x2 banks + avp 3x1 + all dnp rows packed into one
            # bank at base partitions {0, 32, 64} = 8 banks.
            def group_ps(qh):
                return [0, 3, 2, 1] if qh == 0 else [0, 1, 2, 3, 4, 7, 6, 5]

            def head_tasks(h):
                return [(h, qh, p, pos) for qh in (0, 1)
                        for pos, p in enumerate(group_ps(qh))]
            tasks = []
            if interleave2:
                for i in range(NH // 2):
                    for a, b in zip(head_tasks(2 * i), head_tasks(2 * i + 1)):
                        tasks.append(a)
                        tasks.append(b)
            else:
                for h in range(NH):
                    tasks.extend(head_tasks(h))
            state = {}           # (h, qh) -> (avp, dnp)
            prev = None          # (h, qh, p, pt)
            for tsk in tasks + [None]:
                if tsk is not None:
                    h, qh, p, pos = tsk
                    kvh = h // 4
                    qbase = qh * 512
                    if pos == 0:
                        avp_new = ps3.tile([P, 512], F32, tag="av", bufs=2,
                                           name=f"avp{pfx}_{h}_{qh}")
                        dnp_new = ps3.tile([1, 512], F32, tag="dn", bufs=2,
                                           name=f"dnp{pfx}_{h}_{qh}")
                        state[(h, qh)] = (avp_new, dnp_new)
                    qs = max(0, p - 4 * qh) * P
                    stp = ps3.tile([P, 2, 512], F32, tag="st", bufs=2)
                    for jj in range(2):
                        j = 2 * p + jj
                        nc.tensor.matmul(
                            stp[:, jj, qs:512],
                            lhsT=KT[:, kvh, j * P:(j + 1) * P],
                            rhs=QT[:, h, qbase + qs:qbase + 512],
                            start=True, stop=True)
                    pt = p3t.tile([P, 2, 512], BF16, tag="pt", bufs=4)
                    nc.scalar.activation(pt[:, :, qs:512],
                                         stp[:, :, qs:512], AF.Exp)
                    if p >= 4 * qh:      # diagonal junction: causal mask
                        nc.vector.tensor_mul(pt[:, :, qs:qs + P],
                                             pt[:, :, qs:qs + P], mask_sb[:])
                    # pair-sum on DVE halves the PE denominator matmul work
                    pts = p3t.tile([P, 512], BF16, tag="pts", bufs=3)
                    nc.vector.tensor_add(pts[:, qs:512], pt[:, 0, qs:512],
                                         pt[:, 1, qs:512])
                    prev_pts = pts
                if prev is not None:
                    ph, pqh, pp, ppos, ppt, ppts = prev
                    pkvh = ph // 4
                    NP = 4 * (pqh + 1)
                    pqs = max(0, pp - 4 * pqh) * P
                    avp, dnp = state[(ph, pqh)]
                    for jj in range(2):
                        j = 2 * pp + jj
                        nc.tensor.matmul(
                            avp[:, pqs:512],
                            lhsT=Vsb[:, j, pkvh * P:(pkvh + 1) * P],
                            rhs=ppt[:, jj, pqs:512],
                            start=(ppos == 0 and jj == 0),
                            stop=(ppos == NP - 1 and jj == 1))
                    nc.tensor.matmul(
                        dnp[:, pqs:512],
                        lhsT=ones_sb[:],
                        rhs=ppts[:, pqs:512],
                        start=(ppos == 0), stop=(ppos == NP - 1))
                    if ppos == NP - 1:   # last pair of (ph, pqh): drain
                        pqbase = pqh * 512
                        rcp = p3t.tile([1, 512], F32, tag="rcp", bufs=2)
                        scr = p3t.tile([1, 512], F32, tag="scr", bufs=2)
                        nc.vector.reciprocal_approx_accurate(
                            rcp[:], dnp[:], scr[:])
                        rdb = p3t.tile([P, 512], F32, tag="rdb", bufs=2)
                        if bcast == "gpsimd":
                            nc.gpsimd.partition_broadcast(rdb[:], rcp[:])
                        else:
                            # DRAM-bounce broadcast (DMA engines only)
                            rcpd = p3d.tile([1, 512], F32, tag="rcpd", bufs=2)
                            nc.sync.dma_start(rcpd[:], rcp[:])
                            nc.sync.dma_start(
                                rdb[:], rcpd[0:1, :].to_broadcast((P, 512)))
                        nc.vector.tensor_mul(
                            ATT[:, ph, pqbase:pqbase + 512], avp[:], rdb[:])
                        del state[(ph, pqh)]
                prev = (h, qh, p, pos, pt, prev_pts) if tsk is not None else None

        # ================= phase 4: output projection =================
        if 4 in phases:
          with tc.tile_pool(name="p4" + pfx, bufs=1) as p4, \
             tc.tile_pool(name="ps4" + pfx, bufs=1, space="PSUM") as ps4:
            for tt in range(TQ // P):
                for cb in range(2):
                    last = (tt == TQ // P - 1 and cb == 1)
                    if not last:
                        ops = ps4.tile([P, 1024], F32, tag="ops", bufs=2)
                        for htile in range(NH):
                            for nb in range(2):
                                nc.tensor.matmul(
                                    ops[:, nb * 512:(nb + 1) * 512],
                                    lhsT=ATT[:, htile, tt * P:(tt + 1) * P],
                                    rhs=wo_sb[:, htile,
                                              cb * 1024 + nb * 512:cb * 1024 + (nb + 1) * 512],
                                    start=(htile == 0), stop=(htile == NH - 1))
                        osb = p4.tile([P, 1024], F32, tag="osb", bufs=3)
                        nc.scalar.activation(osb[:], ops[:], AF.Copy)
                        nc.sync.dma_start(
                            out[tt * P:(tt + 1) * P, cb * 1024:(cb + 1) * 1024],
                            osb[:])
                    else:
                        # finer tail: the final tile runs as four independent
                        # 256-col accumulation groups so copy+store drain as
                        # each group completes, shortening the post-PE tail
                        for nb in range(4):
                            opsl = ps4.tile([P, 256], F32, tag="opsl", bufs=2)
                            csl = slice(cb * 1024 + nb * 256,
                                        cb * 1024 + (nb + 1) * 256)
                            for htile in range(NH):
                                nc.tensor.matmul(
                                    opsl[:],
                                    lhsT=ATT[:, htile, tt * P:(tt + 1) * P],
                                    rhs=wo_sb[:, htile, csl],
                                    start=(htile == 0), stop=(htile == NH - 1))
                            osb = p4.tile([P, 256], F32, tag="osbl", bufs=4)
                            nc.scalar.activation(osb[:], opsl[:], AF.Copy)
                            nc.sync.dma_start(
                                out[tt * P:(tt + 1) * P, csl], osb[:])


# ---------------------------------------------------------------------------
# host-side wrapper
# ---------------------------------------------------------------------------

_NC = None


def _get_nc():
    global _NC
    if _NC is None:
        _NC = build_program()
    return _NC


def make_in_maps(hidden_states, cos, sin, wq, wk, wv, wo):
    """Build the 8 per-core input dicts (host-side sharding/layout prep).

    Odd-parity cores receive ht/ck/sk with adjacent 128-token blocks swapped
    so their own rows sit in even slots (the SPMD-shared on-device cq/sq
    gather reads even blocks). The swap permutes k-tiles within each pair,
    which only reorders the attention reduction; at the causal junction the
    diagonal lands in the even slot for BOTH parities, with the odd slot
    fully masked (even cores, waste tile) or fully attended (odd cores).
    The 1/sqrt(128) query scale is folded into wqt.
    """
    scale = np.float32(1.0 / np.sqrt(P))

    def tiled(w):
        # [D, N] -> [P, DT, N] (partition-major d-tiles for single-DMA loads)
        wT = np.asarray(w).T
        return np.ascontiguousarray(
            wT.reshape(DT, P, wT.shape[1]).transpose(1, 0, 2)).astype(NPBF16)

    wqt = tiled(np.asarray(wq).astype(np.float64) * scale)
    wkt = tiled(wk)
    wvt = tiled(wv)
    wot = tiled(wo)
    pmat = np.zeros((P, P), np.float32)
    for m in range(64):
        pmat[m + 64, m] = -1.0      # out[m] = -in[m+64]
        pmat[m, m + 64] = 1.0       # out[m+64] = in[m]
    onesc = np.ones((P, 1), NPBF16)
    tri = (np.arange(P)[:, None] <= np.arange(P)[None, :])  # [k, q]: k <= q

    in_maps = []
    for c in range(N_CORES):
        b, r = c // 2, c % 2
        hb = np.asarray(hidden_states[b])                   # [T, D] f32
        own = hb.reshape(T // P, P, D)[r::2].reshape(TQ, D)
        cosb = np.asarray(cos[b])                           # [T, 128]
        sinb = np.asarray(sin[b])
        if r == 1:
            # swap adjacent 128-token blocks so own rows sit in EVEN slots
            swp = np.arange(T // P).reshape(-1, 2)[:, ::-1].reshape(-1)
            hb_k = hb.reshape(T // P, P, D)[swp].reshape(T, D)
            cos_k = cosb.reshape(T // P, P, P)[swp].reshape(T, P)
            sin_k = sinb.reshape(T // P, P, P)[swp].reshape(T, P)
        else:
            hb_k, cos_k, sin_k = hb, cosb, sinb
        maskab_c = np.empty((P, 2, P), np.float32)
        # diagonal in the even slot for both parities; odd slot is the waste
        # tile (even cores) or a fully-attended earlier tile (odd cores)
        maskab_c[:, 0, :] = tri
        maskab_c[:, 1, :] = 0.0 if r == 0 else 1.0
        in_maps.append({
            "ht": tiled(hb_k),
            "hq": tiled(own),
            "wqt": wqt, "wkt": wkt, "wvt": wvt, "wot": wot,
            "ck": np.ascontiguousarray(cos_k.T),
            "sk": np.ascontiguousarray(sin_k.T),
            "maskab": maskab_c.astype(NPBF16),
            "pm": pmat.astype(NPBF16),
            "onesc": onesc,
            "tick": np.zeros((1, 8), np.float32),
        })
    return in_maps


def assemble_output(results):
    out = np.empty((4, T, D), np.float32)
    for c in range(N_CORES):
        b, r = c // 2, c % 2
        out[b].reshape(T // P, P, D)[r::2] = results[c]["out"].reshape(TQ // P, P, D)
    return out


def kernel(hidden_states, cos, sin, wq, wk, wv, wo):
    nc = _get_nc()
    in_maps = make_in_maps(hidden_states, cos, sin, wq, wk, wv, wo)
    res = run_bass_kernel_spmd(nc, in_maps, list(range(N_CORES)))
    return assemble_output(res.results)


if __name__ == "__main__":
    rng = np.random.default_rng(0)
    args = {
        "hidden_states": rng.standard_normal((4, T, D), np.float32),
        "cos": rng.random((4, T, P), np.float32),
        "sin": rng.random((4, T, P), np.float32),
        "wq": rng.standard_normal((NH * P, D), np.float32) / np.sqrt(D),
        "wk": rng.standard_normal((NKV * P, D), np.float32) / np.sqrt(D),
        "wv": rng.standard_normal((NKV * P, D), np.float32) / np.sqrt(D),
        "wo": rng.standard_normal((D, NH * P), np.float32) / np.sqrt(D),
    }
    o = kernel(**args)
    print("ran:", o.shape, o.dtype, np.abs(o).max())
